# revision 15
# baseline (speedup 1.0000x reference)
"""Trainium2 Bass kernel for sparse-attention block (LSH-pooled attention + MLP).

Self-contained: accepts FULL inputs, shards batch across 8 NeuronCores,
returns FULL output. All shapes hardcoded for:
  x [16, 8192, 256], rotations [1, 256, 4, 4], q_w [256,256], kv_w [256,512],
  fc1_w [256,1024], fc2_w [1024,256], norm/bias vectors [256]/[1024].

v6 design notes:
 - Per batch: stage1 (load+LN1 stats), A2 (normalize/transpose/hash/pool),
   kv, B1 (attention), B2 (MLP). Emission interleaves batches:
   B1(b) with stage1(b+1), B2(b) with A2(b+1), so the tensor engine always
   has matmul work and stays at full clock.
 - One activation-table set per phase; rstd = exp(-0.5*ln(var+eps)) batched
   once per batch per layernorm.
 - q-projection folded into keys: scores = (k^T Wq~^T) x^T via a per-batch
   [256,32] effective-key matrix (kills the q matmuls and qt copies).
 - Separate PSUM pools per phase so cross-batch overlap isn't serialized by
   pool-slot rotation.
 - bf16 end-to-end (x converted on host, output upcast on host).
"""

import sys

sys.path.insert(0, "/opt/trn_rl_repo")

from contextlib import ExitStack

import ml_dtypes
import numpy as np

import concourse.bass as bass
import concourse.tile as tile
from concourse import bacc, mybir
from concourse.bass_utils import run_bass_kernel_spmd
from concourse.masks import make_identity

F32 = mybir.dt.float32
BF16 = mybir.dt.bfloat16

N_CORES = 8
B, N, C = 16, 8192, 256
BPC = B // N_CORES          # batches per core
H, DH = 8, 32               # heads
NH, NB = 4, 8               # hashes, buckets
M = NH * NB                 # 32 pooled tokens
DFF = 4 * C                 # 1024
P = 128
TT = N // P                 # 64 token tiles per batch
CH = 512                    # chunk = 4 token tiles
NCHUNK = N // CH            # 16
TPC = CH // P               # 4 tiles per chunk
LN_EPS = 1e-5
AF = mybir.ActivationFunctionType
ALU = mybir.AluOpType


class Emitter:
    def __init__(self, nc, W, pools):
        self.nc = nc
        self.W = W
        (self.sb_chunk, self.sb_cbig, self.sb_hc,
         self.ps_a, self.ps_b, self.ps_mm2, self.ps_acc) = pools
        self.T = {}
        self.kv_state = [None] * BPC
        self.pool_state = [None] * BPC

    # ---------------- stage 1: load + LN1 stats ----------------
    def stage1_chunk(self, xr, c):
        nc, T = self.nc, self.T
        csl = slice(c * TPC, (c + 1) * TPC)
        nc.sync.dma_start(T["XA"][:, csl, :], xr[:, csl, :])
        st = self.sb_chunk.tile([P, TPC, 6], F32, tag="bnst")
        for i in range(TPC):
            t = c * TPC + i
            nc.vector.bn_stats(out=st[:, i], in_=T["XA"][:, t, :])
            nc.vector.bn_aggr(out=T["MV"][:, t, :], in_=st[:, i])

    def stage1_finish(self):
        nc, T, W = self.nc, self.T, self.W
        lnv = self.sb_chunk.tile([P, TT], F32, tag="lnv")
        nc.scalar.activation(lnv[:], T["MV"][:, :, 1], AF.Ln, bias=W["EPS"][:])
        nc.scalar.activation(T["RSD"][:], lnv[:], AF.Exp, scale=-0.5)
        nc.vector.tensor_tensor(T["MRN"][:], T["MV"][:, :, 0], T["RSD"][:], ALU.mult)
        nc.gpsimd.tensor_scalar_mul(T["MRN"][:], T["MRN"][:], -1.0)

    # ---------------- A2: normalize, transpose, hash, pool ----------------
    def a2_chunk(self, b, c):
        nc, T, W = self.nc, self.T, self.W
        IDENT = W["IDENT"]
        if c == 0:
            self.pool_state[b] = self.ps_acc.tile([M, 512], F32, tag="acc", name="ps_pool")
        ps_pool = self.pool_state[b]
        for i in range(TPC):
            t = c * TPC + i
            if t % 2 == 0:
                nc.vector.tensor_scalar(
                    out=T["X_"][:, t, 0:C],
                    in0=T["XA"][:, t, :],
                    scalar1=T["MV"][:, t, 0:1],
                    scalar2=T["RSD"][:, t : t + 1],
                    op0=ALU.subtract,
                    op1=ALU.mult,
                )
            else:
                nc.scalar.activation(
                    T["X_"][:, t, 0:C], T["XA"][:, t, :], AF.Identity,
                    bias=T["MRN"][:, t : t + 1], scale=T["RSD"][:, t : t + 1],
                )
            pst = self.ps_a.tile([P, 2, P], BF16, tag="psta")
            for h in range(2):
                nc.tensor.transpose(pst[:, h, :], T["X_"][:, t, h * P : (h + 1) * P], IDENT[:])
            nc.vector.tensor_copy(T["XT"][:, :, t, :], pst[:])
            psr_t = self.ps_a.tile([P, 2, P], F32, tag="psta")
            psr = psr_t[:, 0, 0:16]
            nc.tensor.matmul(psr, T["XT"][:, 0, t, :], W["ROT"][:, 0, :], start=True, stop=False)
            nc.tensor.matmul(psr, T["XT"][:, 1, t, :], W["ROT"][:, 1, :], start=False, stop=True)
            rt = self.sb_chunk.tile([P, NH, NH], F32, tag="rt")
            nc.vector.tensor_copy(rt[:], psr.rearrange("p (h i) -> p h i", h=NH))
            am = self.sb_chunk.tile([P, NH], F32, tag="am")
            nc.vector.tensor_reduce(
                out=am[:], in_=rt[:], axis=mybir.AxisListType.X,
                op=ALU.max, apply_absolute_value=True,
            )
            nam = self.sb_chunk.tile([P, NH], F32, tag="nam")
            nc.gpsimd.tensor_scalar_mul(nam[:], am[:], -1.0)
            oh = self.sb_chunk.tile([P, NH, NB], BF16, tag="oh")
            nc.vector.tensor_tensor(
                oh[:, :, 0:NH], rt[:], am[:, :, None].to_broadcast((P, NH, NH)),
                ALU.is_equal,
            )
            nc.vector.tensor_tensor(
                oh[:, :, NH:NB], rt[:], nam[:, :, None].to_broadcast((P, NH, NH)),
                ALU.is_equal,
            )
            nc.tensor.matmul(
                ps_pool[:, 0 : C + 1],
                oh[:].rearrange("p h b -> p (h b)"),
                T["X_"][:, t, 0 : C + 1],
                start=(t == 0), stop=(t == TT - 1), skip_group_check=True,
            )

    # ---------------- kv: pooled -> effective keys + vhat ----------------
    def kv_section(self, b):
        nc, T, W = self.nc, self.T, self.W
        IDENT = W["IDENT"]
        sb = self.sb_chunk
        ps_pool = self.pool_state[b]

        pcb = sb.tile([M, C], BF16, tag="pcb")
        nc.vector.tensor_copy(pcb[:], ps_pool[:, 0:C])
        invc = sb.tile([M, 1], F32, tag="invc")
        nc.vector.tensor_scalar_add(invc[:], ps_pool[:, C : C + 1], 1e-20)
        nc.vector.reciprocal(invc[:], invc[:])
        ptb = sb.tile([P, 2, M], BF16, tag="ptb")
        pstp = self.ps_b.tile([P, 2, P], BF16, tag="pstb")
        for h in range(2):
            nc.tensor.transpose(pstp[:, h, 0:M], pcb[:, h * P : (h + 1) * P], IDENT[:M, :M])
        nc.vector.tensor_copy(ptb[:], pstp[:, :, 0:M])
        pskv = self.ps_mm2.tile([M, 2 * C], F32, tag="mm2")
        nc.tensor.matmul(pskv[:], ptb[:, 0, :], W["WKV"][:, 0, :], start=True, stop=False)
        nc.tensor.matmul(pskv[:], ptb[:, 1, :], W["WKV"][:, 1, :], start=False, stop=True)
        kv = sb.tile([M, 2 * C], BF16, tag="kv")
        nc.vector.tensor_scalar_mul(kv[:], pskv[:], invc[:])
        khat = sb.tile([P, 2, P], BF16, tag="khat")
        vhat = sb.tile([P, 2, P], BF16, tag="vhat")
        nc.vector.memset(khat[:], 0.0)
        nc.vector.memset(vhat[:], 0.0)
        for h2 in range(2):
            pskt_t = self.ps_b.tile([P, 2, P], BF16, tag="pstb")
            pskt = pskt_t[:, 0]
            nc.tensor.transpose(pskt[:, 0:M], kv[:, h2 * P : (h2 + 1) * P], IDENT[:M, :M])
            for j in range(4):
                nc.vector.tensor_copy(
                    khat[32 * j : 32 * (j + 1), h2, 32 * j : 32 * (j + 1)],
                    pskt[32 * j : 32 * (j + 1)][:, 0:M],
                )
                nc.gpsimd.tensor_copy(
                    vhat[32 * j : 32 * (j + 1), h2, 32 * j : 32 * (j + 1)],
                    kv[:, C + h2 * P + 32 * j : C + h2 * P + 32 * (j + 1)],
                )
        # effective keys: KET[m', c] = sum_d k[m', d] * wq_scaled[c, 32h+d]
        psket = self.ps_mm2.tile([P, 2, C], F32, tag="mm2")
        for h2 in range(2):
            nc.tensor.matmul(
                psket[:, h2, :], khat[:, h2, :], W["WQT"][:, h2, :],
                start=True, stop=True, skip_group_check=True,
            )
        kes = sb.tile([P, 2, C], BF16, tag="kes")
        nc.vector.tensor_copy(kes[:], psket[:])
        KEH = sb.tile([P, 2, 2, P], BF16, tag="keh")
        for k2 in range(2):
            psket_t = self.ps_b.tile([P, 2, P], BF16, tag="pstb")
            for h2 in range(2):
                nc.tensor.transpose(
                    psket_t[:, h2, :], kes[:, h2, k2 * P : (k2 + 1) * P], IDENT[:]
                )
            nc.vector.tensor_copy(KEH[:, k2, :, :], psket_t[:])
        self.kv_state[b] = (KEH, vhat)

    # ---------------- B1: attention chunk ----------------
    def b1_chunk(self, b, c):
        nc, T, W = self.nc, self.T, self.W
        IDENT = W["IDENT"]
        KEH, vhat = self.kv_state[b]
        csl = slice(c * TPC, (c + 1) * TPC)
        xb2 = T["XA"][:, csl, :]
        psa = self.ps_mm2.tile([P, 2, CH], F32, tag="mm2")
        for h2 in range(2):
            nc.tensor.matmul(
                psa[:, h2, :], KEH[:, 0, h2, :], T["XT"][:, 0, csl, :],
                start=True, stop=False, skip_group_check=True,
            )
            nc.tensor.matmul(
                psa[:, h2, :], KEH[:, 1, h2, :], T["XT"][:, 1, csl, :],
                start=False, stop=True, skip_group_check=True,
            )
        expc = self.sb_chunk.tile([P, 2, CH], BF16, tag="expc")
        nc.scalar.activation(expc[:], psa[:], AF.Exp)
        psz = self.ps_acc.tile([H, CH], F32, tag="acc", name="psz")
        for h2 in range(2):
            nc.tensor.matmul(
                psz[:], W["SB8"][:, h2, :], expc[:, h2, :],
                start=(h2 == 0), stop=(h2 == 1), skip_group_check=True,
            )
        zsb = self.sb_chunk.tile([H, CH], BF16, tag="zsb")
        nc.vector.tensor_copy(zsb[:], psz[:])
        psznat_t = self.ps_b.tile([P, 2, P], BF16, tag="pstb")
        psznat = psznat_t[:].rearrange("p a b -> p (a b)")[:, 0 : TPC * H].rearrange(
            "p (i h) -> p i h", h=H
        )
        for i in range(TPC):
            nc.tensor.transpose(psznat[:, i, :], zsb[:, i * P : (i + 1) * P], IDENT[:H, :H])
        nc.vector.reciprocal(out=T["ZN"][:, csl, :], in_=psznat)
        pso = self.ps_mm2.tile([P, 2, CH], F32, tag="mm2")
        for h2 in range(2):
            nc.tensor.matmul(
                pso[:, h2, :], vhat[:, h2, :], expc[:, h2, :],
                start=True, stop=True, skip_group_check=True,
            )
        ot = self.sb_chunk.tile([P, 2, CH], BF16, tag="ot")
        nc.scalar.activation(ot[:], pso[:], AF.Copy)
        st2 = self.sb_chunk.tile([P, TPC, 6], F32, tag="bnst2")
        for i in range(TPC):
            t = c * TPC + i
            psn = self.ps_b.tile([P, 2, P], BF16, tag="pstb")
            for h2 in range(2):
                nc.tensor.transpose(psn[:, h2, :], ot[:, h2, i * P : (i + 1) * P], IDENT[:])
            tmp = self.sb_chunk.tile([P, H, DH], BF16, tag="tmp")
            nc.vector.tensor_tensor(
                tmp[:],
                psn[:].rearrange("p a b -> p (a b)").rearrange("p (h d) -> p h d", h=H),
                T["ZN"][:, t, :, None].to_broadcast((P, H, DH)),
                ALU.mult,
            )
            nc.vector.tensor_tensor(
                T["X2"][:, t, :], tmp[:].rearrange("p h d -> p (h d)"), xb2[:, i, :],
                ALU.add,
            )
            nc.vector.bn_stats(out=st2[:, i], in_=T["X2"][:, t, :])
            nc.vector.bn_aggr(out=T["MV2"][:, t, :], in_=st2[:, i])

    def b1_finish(self):
        nc, T, W = self.nc, self.T, self.W
        lnv2 = self.sb_chunk.tile([P, TT], F32, tag="lnv2")
        nc.scalar.activation(lnv2[:], T["MV2"][:, :, 1], AF.Ln, bias=W["EPS"][:])
        nc.scalar.activation(T["RSD2"][:], lnv2[:], AF.Exp, scale=-0.5)
        nc.vector.tensor_tensor(T["MRN2"][:], T["MV2"][:, :, 0], T["RSD2"][:], ALU.mult)
        nc.gpsimd.tensor_scalar_mul(T["MRN2"][:], T["MRN2"][:], -1.0)

    # ---------------- B2: MLP chunk ----------------
    def b2_chunk(self, b, orr, c):
        nc, T, W = self.nc, self.T, self.W
        IDENT = W["IDENT"]
        csl = slice(c * TPC, (c + 1) * TPC)
        yt = self.sb_chunk.tile([P, 2, TPC, P], BF16, tag="yt")
        for i in range(TPC):
            t = c * TPC + i
            yc = self.sb_chunk.tile([P, C], BF16, tag="yc")
            if t % 2 == 0:
                nc.vector.tensor_scalar(
                    out=yc[:], in0=T["X2"][:, t, :],
                    scalar1=T["MV2"][:, t, 0:1], scalar2=T["RSD2"][:, t : t + 1],
                    op0=ALU.subtract, op1=ALU.mult,
                )
            else:
                nc.scalar.activation(
                    yc[:], T["X2"][:, t, :], AF.Identity,
                    bias=T["MRN2"][:, t : t + 1], scale=T["RSD2"][:, t : t + 1],
                )
            psy_t = self.ps_b.tile([P, 2, P], BF16, tag="pstb")
            for h in range(2):
                nc.tensor.transpose(psy_t[:, h, :], yc[:, h * P : (h + 1) * P], IDENT[:])
            nc.vector.tensor_copy(yt[:, :, i, :], psy_t[:])
        ytf = yt[:].rearrange("p k i q -> p k (i q)")
        hc = self.sb_hc.tile([P, 8, CH], BF16, tag="hc")
        for mp in range(4):
            psh = self.ps_mm2.tile([P, 2, CH], F32, tag="mm2")
            for mi in range(2):
                m = 2 * mp + mi
                nc.tensor.matmul(
                    psh[:, mi, :], W["W1"][:, 0, m * P : (m + 1) * P], ytf[:, 0, :],
                    start=True, stop=False, skip_group_check=True,
                )
                nc.tensor.matmul(
                    psh[:, mi, :], W["W1"][:, 1, m * P : (m + 1) * P], ytf[:, 1, :],
                    start=False, stop=True, skip_group_check=True,
                )
                nc.scalar.activation(
                    hc[:, m, :], psh[:, mi, :], AF.Gelu, bias=W["B1T"][:, m : m + 1],
                )
        psy = self.ps_mm2.tile([P, 2, CH], F32, tag="mm2")
        for m2 in range(2):
            for k in range(8):
                nc.tensor.matmul(
                    psy[:, m2, :],
                    W["W2"][:, k, m2 * P : (m2 + 1) * P],
                    hc[:, k, :],
                    start=(k == 0), stop=(k == 7),
                    skip_group_check=True,
                )
        yo = self.sb_chunk.tile([P, 2, CH], BF16, tag="yo")
        for m2 in range(2):
            nc.scalar.activation(
                yo[:, m2, :], psy[:, m2, :], AF.Identity, bias=W["B2T"][:, m2 : m2 + 1]
            )
        outc = self.sb_cbig.tile([P, TPC, C], BF16, tag="outc")
        for i in range(TPC):
            t = c * TPC + i
            psm = self.ps_b.tile([P, 2, P], BF16, tag="pstb")
            for h2 in range(2):
                nc.tensor.transpose(psm[:, h2, :], yo[:, h2, i * P : (i + 1) * P], IDENT[:])
            nc.vector.tensor_tensor(
                outc[:, i, :], psm[:].rearrange("p a b -> p (a b)"), T["X2"][:, t, :],
                ALU.add,
            )
        nc.sync.dma_start(orr[:, csl, :], outc[:])


def _build(affine_flags, repeat=1):
    assert not any(affine_flags), "affine path not implemented"
    nc = bacc.Bacc("TRN2", target_bir_lowering=False, debug=False, enable_asserts=True)

    x_ap = nc.dram_tensor("x", [BPC, N, C], BF16, kind="ExternalInput").ap()
    wqt = nc.dram_tensor("wqt", [C, C], BF16, kind="ExternalInput").ap()
    wkv = nc.dram_tensor("wkv", [C, 2 * C], BF16, kind="ExternalInput").ap()
    rot = nc.dram_tensor("rot", [C, 16], BF16, kind="ExternalInput").ap()
    w1 = nc.dram_tensor("w1", [C, DFF], BF16, kind="ExternalInput").ap()
    w2 = nc.dram_tensor("w2", [DFF, C], BF16, kind="ExternalInput").ap()
    b1t = nc.dram_tensor("b1t", [P, 8], F32, kind="ExternalInput").ap()
    b2t = nc.dram_tensor("b2t", [P, 2], F32, kind="ExternalInput").ap()
    o_ap = nc.dram_tensor("out", [BPC, N, C], BF16, kind="ExternalOutput").ap()

    with tile.TileContext(nc) as tc:
        with ExitStack() as ctx:
            sb_w = ctx.enter_context(tc.tile_pool(name="weights", bufs=1))
            sb_trunk = ctx.enter_context(tc.tile_pool(name="trunk", bufs=1))
            sb_chunk = ctx.enter_context(tc.tile_pool(name="chunk", bufs=3))
            sb_cbig = ctx.enter_context(tc.tile_pool(name="cbig", bufs=2))
            sb_hc = ctx.enter_context(tc.tile_pool(name="hc", bufs=1))
            ps_a = ctx.enter_context(tc.tile_pool(name="ps_a", bufs=1, space="PSUM"))
            ps_b = ctx.enter_context(tc.tile_pool(name="ps_b", bufs=2, space="PSUM"))
            ps_mm2 = ctx.enter_context(tc.tile_pool(name="ps_mm2", bufs=2, space="PSUM"))
            ps_acc = ctx.enter_context(tc.tile_pool(name="ps_acc", bufs=1, space="PSUM"))

            W = {}
            W["IDENT"] = sb_w.tile([P, P], BF16, name="IDENT")
            make_identity(nc, W["IDENT"][:])
            W["WQT"] = sb_w.tile([P, 2, C], BF16, name="WQT")
            nc.sync.dma_start(W["WQT"][:], wqt.rearrange("(h p) c -> p h c", p=P))
            W["WKV"] = sb_w.tile([P, 2, 2 * C], BF16, name="WKV")
            nc.sync.dma_start(W["WKV"][:], wkv.rearrange("(k p) m -> p k m", p=P))
            W["ROT"] = sb_w.tile([P, 2, 16], BF16, name="ROTW")
            nc.sync.dma_start(W["ROT"][:], rot.rearrange("(k p) m -> p k m", p=P))
            W["W1"] = sb_w.tile([P, 2, DFF], BF16, name="W1")
            nc.sync.dma_start(W["W1"][:], w1.rearrange("(k p) m -> p k m", p=P))
            W["W2"] = sb_w.tile([P, 8, C], BF16, name="W2")
            nc.sync.dma_start(W["W2"][:], w2.rearrange("(k p) m -> p k m", p=P))
            W["B1T"] = sb_w.tile([P, 8], F32, name="B1T")
            nc.sync.dma_start(W["B1T"][:], b1t)
            W["B2T"] = sb_w.tile([P, 2], F32, name="B2T")
            nc.sync.dma_start(W["B2T"][:], b2t)
            W["EPS"] = sb_w.tile([P, 1], F32, name="EPS")
            nc.vector.memset(W["EPS"][:], LN_EPS)
            W["SB8"] = sb_w.tile([P, 2, 8], BF16, name="SB8")
            nc.vector.memset(W["SB8"][:], 0.0)
            for h2 in range(2):
                for jl in range(4):
                    nc.vector.memset(
                        W["SB8"][32 * jl : 32 * (jl + 1), h2, h2 * 4 + jl : h2 * 4 + jl + 1], 1.0
                    )

            em = Emitter(nc, W, (sb_chunk, sb_cbig, sb_hc, ps_a, ps_b, ps_mm2, ps_acc))
            T = em.T
            T["XA"] = sb_trunk.tile([P, TT, C], BF16, name="XA")
            T["X_"] = sb_trunk.tile([P, TT, C + 2], BF16, name="Xn")
            T["X2"] = sb_trunk.tile([P, TT, C], BF16, name="X2")
            T["XT"] = sb_trunk.tile([P, 2, TT, P], BF16, name="XT")
            T["MV"] = sb_trunk.tile([P, TT, 2], F32, name="MV")
            T["RSD"] = sb_trunk.tile([P, TT], F32, name="RSD")
            T["MV2"] = sb_trunk.tile([P, TT, 2], F32, name="MV2")
            T["RSD2"] = sb_trunk.tile([P, TT], F32, name="RSD2")
            T["ZN"] = sb_trunk.tile([P, TT, H], F32, name="ZN")
            T["MRN"] = sb_trunk.tile([P, TT], F32, name="MRN")
            T["MRN2"] = sb_trunk.tile([P, TT], F32, name="MRN2")
            nc.vector.memset(T["X_"][:, :, C : C + 1], 1.0)

            xrs = [x_ap[b].rearrange("(t p) c -> p t c", p=P) for b in range(BPC)]
            orrs = [o_ap[b].rearrange("(t p) c -> p t c", p=P) for b in range(BPC)]

            # software pipeline across batches:
            #   stage1(0); A2(0); kv(0);
            #   for b: [B1(b,c) | stage1(b+1,c)]; finishes;
            #          [B2(b,c) | A2(b+1,c)]; kv(b+1)
            for _r in range(repeat):
                for c in range(NCHUNK):
                    em.stage1_chunk(xrs[0], c)
                em.stage1_finish()
                for c in range(NCHUNK):
                    em.a2_chunk(0, c)
                em.kv_section(0)
                for b in range(BPC):
                    nb = b + 1
                    for c in range(NCHUNK):
                        em.b1_chunk(b, c)
                        if nb < BPC:
                            em.stage1_chunk(xrs[nb], c)
                    em.b1_finish()
                    if nb < BPC:
                        em.stage1_finish()
                    for c in range(NCHUNK):
                        em.b2_chunk(b, orrs[b], c)
                        if nb < BPC:
                            em.a2_chunk(nb, c)
                    if nb < BPC:
                        em.kv_section(nb)

    nc.compile()
    return nc


_NC_CACHE = {}


def _get_nc(affine_flags, repeat=1):
    key = (affine_flags, repeat)
    if key not in _NC_CACHE:
        _NC_CACHE[key] = _build(affine_flags, repeat)
    return _NC_CACHE[key]


def make_in_maps(x, rotations, q_w, kv_w, fc1_w, fc2_w, fc1_b, fc2_b):
    bf = ml_dtypes.bfloat16
    scale = DH ** -0.5
    common = {
        "wqt": np.ascontiguousarray((np.asarray(q_w, np.float32) * scale).T).astype(bf),
        "wkv": np.asarray(kv_w, np.float32).astype(bf),
        "rot": np.asarray(rotations, np.float32).reshape(C, NH * (NB // 2)).astype(bf),
        "w1": np.asarray(fc1_w, np.float32).astype(bf),
        "w2": np.asarray(fc2_w, np.float32).astype(bf),
        "b1t": np.ascontiguousarray(np.asarray(fc1_b, np.float32).reshape(8, P).T),
        "b2t": np.ascontiguousarray(np.asarray(fc2_b, np.float32).reshape(2, P).T),
    }
    xs = np.asarray(x, np.float32).astype(bf).reshape(N_CORES, BPC, N, C)
    return [{**common, "x": np.ascontiguousarray(xs[i])} for i in range(N_CORES)]


def kernel(
    x, rotations, norm1_g, norm1_b, q_w, kv_w, norm2_g, norm2_b,
    fc1_w, fc1_b, fc2_w, fc2_b,
):
    use_g1 = not np.allclose(np.asarray(norm1_g), 1.0)
    use_b1 = not np.allclose(np.asarray(norm1_b), 0.0)
    use_g2 = not np.allclose(np.asarray(norm2_g), 1.0)
    use_b2 = not np.allclose(np.asarray(norm2_b), 0.0)
    flags = (use_g1, use_b1, use_g2, use_b2)
    nc = _get_nc(flags)

    in_maps = make_in_maps(x, rotations, q_w, kv_w, fc1_w, fc2_w, fc1_b, fc2_b)
    res = run_bass_kernel_spmd(nc, in_maps, core_ids=list(range(N_CORES)))
    out = np.concatenate(
        [res.results[i]["out"].astype(np.float32) for i in range(N_CORES)], axis=0
    )
    return out.reshape(B, N, C)
